# revision 25
# baseline (speedup 1.0000x reference)
"""Trainium2 Bass kernel for BiLinearLayerV2.

  biLinear[b,f,d] = sum_e feature[b,f,e] * weight[f,e,d]
  out[b,f,g,d]    = biLinear[b,f,d] * feature[b,g,d] * weightLeft[f,g]

Shapes: feature [512,64,32] f32, weight [64,32,32], weightLeft [64,64]
Output: [512,64,64,32] f32 (256 MB) -> heavily store-bound.

Data-parallel over batch (64 per core x 8 cores). Per core the output is
33.5 MB, so the kernel lives at the HBM-store roofline; every other byte
of traffic must be squeezed out. Two-stage structure ("kron" design):

  1. biLinT_f[d,b] = sum_e weight[f,e,d] * feature[b,f,e] -- computed
     on-chip in fp32 (exact w.r.t. the cancellation-prone e-sum), tiny.
  2. Q_f[b,(g,d)] = biLinT_f[d,b] * wL[f,g] via PE matmul with moving
     operand R_f[d',(g,d)] = mask(d'==d) * wL[f,g], built ON-CHIP per
     f-group as (constant 0/1 mask) * (broadcast wL row). Both operands
     fp16: stage 2 is a pure product, so fp16 rounding enters
     multiplicatively (no cancellation blow-up) -> rel err ~1e-3.
  3. DVE: out = (Q / 2^15) * feature  (PSUM -> SBUF), stores via HWDGE.

This removes the 16.8 MB fused-weight (wv) HBM load of the naive scheme;
per-core DMA is ~2 MB of inputs + 33.5 MB of stores.

fp16 subnormal guard: wL is pre-scaled by 2^10 and biLinT by 2^5 so all
fp16-carried values stay in the normal range; the final DVE pass undoes
the 2^15.
"""

import sys

if "/opt/trn_rl_repo" not in sys.path:
    sys.path.insert(0, "/opt/trn_rl_repo")

import numpy as np

B, F, E = 512, 64, 32
NCORES = 8
BLOC = B // NCORES  # 64
GD = F * E  # 2048
SCALE_WL = 1024.0  # 2^10 on weightLeft
SCALE_BL = 32.0  # 2^5 on biLinT (fp16 cast)
DESCALE = 1.0 / (SCALE_WL * SCALE_BL)

# engine for the on-chip R = mask * wL broadcast build: "vector" | "gpsimd"
RBUILD = "gpsimd"

_cached = {}


def _build_nc(rbuild=RBUILD):
    from contextlib import ExitStack

    import concourse.bass as bass
    import concourse.tile as tile
    from concourse import bacc, mybir

    f32 = mybir.dt.float32
    f16 = mybir.dt.float16
    nc = bacc.Bacc("TRN2", target_bir_lowering=False, debug=False)

    # packed inputs (fp32): rows 32*q+e hold f-quarter q = f//16:
    #   featT4[32q+e, (f%16)*64+b] | wstat4[32q+e, (f%16)*32+d]
    c32 = nc.dram_tensor(
        "c32", (128, 16 * BLOC + 16 * E), f32, kind="ExternalInput"
    ).ap()
    c16 = nc.dram_tensor(
        "c16", (128, 16 * F + 2 * GD), f16, kind="ExternalInput"
    ).ap()
    out = nc.dram_tensor("out", (BLOC, F, GD), f32, kind="ExternalOutput").ap()

    with tile.TileContext(nc) as tc, ExitStack() as ctx:
        consts = ctx.enter_context(tc.tile_pool(name="consts", bufs=1))
        # phase-A fp32 operands on sync; fp16 consts on scalar -- 2 DMAs total
        c32_t = consts.tile([128, 16 * BLOC + 16 * E], f32)
        nc.sync.dma_start(c32_t[:], c32)
        c16_t = consts.tile([128, 16 * F + 2 * GD], f16)
        # R-build consts (wlrep|mask) on their own queue so R0 starts early;
        # featd is only needed once the first STT runs
        nc.scalar.dma_start(
            c16_t[:, 0 : 16 * F + GD], c16[:, 0 : 16 * F + GD]
        )
        nc.sync.dma_start(
            c16_t[:, 16 * F + GD :], c16[:, 16 * F + GD :]
        )
        featT_t = c32_t[:, 0 : 16 * BLOC]
        wstat_t = c32_t[:, 16 * BLOC : 16 * BLOC + 16 * E]
        wlrep_t = c16_t[:, 0 : 16 * F]
        mask_t = c16_t[:, 16 * F : 16 * F + GD]
        featd_t = c16_t[:, 16 * F + GD : 16 * F + 2 * GD]
        # biLinT16[(r,d'), j4*64+b] = biLinear[b, 4*j4+r, d'] * 2^5, fp16
        bilin_t = consts.tile([128, 16 * BLOC], f16)

        # Phase A: biLinT via fp32 matmuls, 4 f's concurrent in col-groups.
        with tc.tile_pool(name="psb", bufs=2, space=bass.MemorySpace.PSUM) as psb:
            for j4 in range(16):
                bps = psb.tile([128, BLOC], f32)
                for r in range(4):
                    f = 4 * j4 + r
                    q, fq = f // 16, f % 16
                    qsl = slice(32 * q, 32 * q + 32)
                    nc.tensor.matmul(
                        bps[32 * r : 32 * r + 32, :],
                        wstat_t[qsl, 32 * fq : 32 * fq + 32],
                        featT_t[qsl, fq * BLOC : (fq + 1) * BLOC],
                        start=True,
                        stop=True,
                        tile_position=(32 * q, 32 * r),
                    )
                # cast to fp16 with 2^5 pre-scale (ACT engine; keeps DVE free)
                nc.scalar.activation(
                    bilin_t[:, j4 * BLOC : (j4 + 1) * BLOC],
                    bps[:],
                    mybir.ActivationFunctionType.Copy,
                    scale=SCALE_BL,
                )

        reng = nc.vector if rbuild == "vector" else nc.gpsimd
        with (
            tc.tile_pool(name="rt", bufs=4) as rtp,
            tc.tile_pool(name="psq", bufs=2, space=bass.MemorySpace.PSUM) as psq,
            tc.tile_pool(name="ot", bufs=6) as otp,
        ):
            mask3 = mask_t.rearrange("p (g d) -> p g d", d=E)
            for j4 in range(16):
                # R[(r,d'),(g,d)] = mask(d'==d) * wL[4j4+r, g] * 2^10  (fp16)
                rt = rtp.tile([128, GD], f16)
                wl3 = (
                    wlrep_t[:, j4 * F : (j4 + 1) * F]
                    .unsqueeze(2)
                    .broadcast_to((128, F, E))
                )
                # first two R tiles on the (startup-idle) DVE so j4=0's
                # matmuls don't wait for gpsimd's slower first build
                reng = nc.vector if j4 < 2 else (
                    nc.vector if rbuild == "vector" else nc.gpsimd
                )
                reng.tensor_tensor(
                    rt[:].rearrange("p (g d) -> p g d", d=E),
                    mask3,
                    wl3,
                    mybir.AluOpType.mult,
                )
                # half-tile (1024-col) PSUM units: pq tags pq0/pq1 x bufs=2
                # ping-pong across cc -> PE never waits on the DVE drain.
                # Both cc halves land in one [128,2048] SBUF tile so stores
                # stay 512 KB with 8 KB contiguous runs.
                for s in range(2):
                    f0 = 4 * j4 + 2 * s
                    # stores round-robin over three DMA queues
                    eng = (nc.sync, nc.scalar, nc.gpsimd)[(2 * j4 + s) % 3]
                    ot = otp.tile([128, GD], f32)
                    for cc in range(2):
                        csl = slice(1024 * cc, 1024 * (cc + 1))
                        pq = psq.tile([128, 1024], f32, name=f"pq{s}", tag=f"pq{s}")
                        for n in range(2):
                            nsl = slice(1024 * cc + 512 * n, 1024 * cc + 512 * (n + 1))
                            for half in range(2):
                                r = 2 * s + half
                                rsl = slice(32 * r, 32 * r + 32)
                                nc.tensor.matmul(
                                    pq[64 * half : 64 * half + 64, 512 * n : 512 * (n + 1)],
                                    bilin_t[rsl, j4 * BLOC : (j4 + 1) * BLOC],
                                    rt[rsl, nsl],
                                    start=True,
                                    stop=True,
                                    tile_position=(32 * r, 64 * half),
                                )
                        # out = (Q / 2^15) * feature
                        nc.vector.scalar_tensor_tensor(
                            ot[:, csl],
                            pq[:],
                            DESCALE,
                            featd_t[:, csl],
                            op0=mybir.AluOpType.mult,
                            op1=mybir.AluOpType.mult,
                        )
                    eng.dma_start(out[:, f0, :], ot[0:64, :])
                    eng.dma_start(out[:, f0 + 1, :], ot[64:128, :])

    nc.compile()
    return nc


def _get_nc(rbuild=RBUILD):
    if rbuild not in _cached:
        _cached[rbuild] = _build_nc(rbuild)
    return _cached[rbuild]


def _host_inputs(feature, weight, weightLeft, rbuild=RBUILD):
    """Per-core input maps. Host work is layout prep of weights/inputs only."""
    feature = np.ascontiguousarray(feature, dtype=np.float32)
    weight = np.ascontiguousarray(weight, dtype=np.float32)
    weightLeft = np.ascontiguousarray(weightLeft, dtype=np.float32)

    # wstat[e, f*32+d] = weight[f,e,d]
    wstat = np.ascontiguousarray(weight.transpose(1, 0, 2).reshape(E, F * E))
    # wlrep[32*r+d', j4*64+g] = wL[4*j4+r, g] * 2^10   (d'-replicated)
    wl4 = (weightLeft * np.float32(SCALE_WL)).reshape(16, 4, F)  # [j4, r, g]
    wlrep = np.broadcast_to(
        wl4.transpose(1, 0, 2)[:, None, :, :], (4, E, 16, F)
    ).reshape(128, 16 * F)
    wlrep = np.ascontiguousarray(wlrep.astype(np.float16))
    # maskc[32*r+d', g*32+d] = 1 if d==d' else 0
    maskc = np.ascontiguousarray(np.tile(np.eye(E, dtype=np.float16), (4, F)))

    # f-quarter packing: rows 32*q+e for q = f//16
    wstat4 = (
        wstat.reshape(E, 4, 16 * E).transpose(1, 0, 2).reshape(128, 16 * E)
    )
    in_maps = []
    for c in range(NCORES):
        fc = feature[c * BLOC : (c + 1) * BLOC]  # [64, 64, 32]
        featd = np.tile(fc.reshape(BLOC, GD), (2, 1)).astype(np.float16)
        featT = fc.transpose(2, 1, 0).reshape(E, F * BLOC)
        featT4 = (
            featT.reshape(E, 4, 16 * BLOC)
            .transpose(1, 0, 2)
            .reshape(128, 16 * BLOC)
        )
        c32 = np.ascontiguousarray(np.concatenate([featT4, wstat4], axis=1))
        c16 = np.ascontiguousarray(
            np.concatenate([wlrep, maskc, featd], axis=1)
        )
        in_maps.append({"c32": c32, "c16": c16})
    return in_maps


def _run(in_maps, trace=False, tmpdir=None, rbuild=RBUILD):
    from concourse.bass_utils import run_bass_kernel_spmd

    nc = _get_nc(rbuild)
    return run_bass_kernel_spmd(
        nc, in_maps, core_ids=list(range(NCORES)), trace=trace, tmpdir=tmpdir
    )


def kernel(feature, weight, weightLeft):
    in_maps = _host_inputs(feature, weight, weightLeft)
    res = _run(in_maps)
    out = np.concatenate(
        [r["out"].reshape(BLOC, F, F, E) for r in res.results], axis=0
    )
    return out


# revision 29
# speedup vs baseline: 1.0972x; 1.0972x over previous
"""Trainium2 Bass kernel for BiLinearLayerV2.

  biLinear[b,f,d] = sum_e feature[b,f,e] * weight[f,e,d]
  out[b,f,g,d]    = biLinear[b,f,d] * feature[b,g,d] * weightLeft[f,g]

Shapes: feature [512,64,32] f32, weight [64,32,32], weightLeft [64,64]
Output: [512,64,64,32] f32 (256 MB) -> heavily store-bound.

Data-parallel over batch (64 per core x 8 cores). Per core the output is
33.5 MB, so the kernel lives at the HBM-store roofline; every other byte
of traffic must be squeezed out. Two-stage structure ("kron" design):

  1. biLinT_f[d,b] = sum_e weight[f,e,d] * feature[b,f,e] -- computed
     on-chip in fp32 (exact w.r.t. the cancellation-prone e-sum), tiny.
  2. Q_f[b,(g,d)] = biLinT_f[d,b] * wL[f,g] via PE matmul with moving
     operand R_f[d',(g,d)] = mask(d'==d) * wL[f,g], built ON-CHIP per
     f-group as (constant 0/1 mask) * (broadcast wL row). Both operands
     fp16: stage 2 is a pure product, so fp16 rounding enters
     multiplicatively (no cancellation blow-up) -> rel err ~1e-3.
  3. DVE: out = (Q / 2^15) * feature  (PSUM -> SBUF), stores via HWDGE.

This removes the 16.8 MB fused-weight (wv) HBM load of the naive scheme;
per-core DMA is ~2 MB of inputs + 33.5 MB of stores.

fp16 subnormal guard: wL is pre-scaled by 2^10 and biLinT by 2^5 so all
fp16-carried values stay in the normal range; the final DVE pass undoes
the 2^15.
"""

import sys

if "/opt/trn_rl_repo" not in sys.path:
    sys.path.insert(0, "/opt/trn_rl_repo")

import numpy as np

B, F, E = 512, 64, 32
NCORES = 8
BLOC = B // NCORES  # 64
GD = F * E  # 2048
SCALE_WL = 1024.0  # 2^10 on weightLeft
SCALE_BL = 32.0  # 2^5 on biLinT (fp16 cast)
DESCALE = 1.0 / (SCALE_WL * SCALE_BL)

# engine for the on-chip R = mask * wL broadcast build: "vector" | "gpsimd"
RBUILD = "gpsimd"

_cached = {}


def _build_nc(rbuild=RBUILD):
    from contextlib import ExitStack

    import concourse.bass as bass
    import concourse.tile as tile
    from concourse import bacc, mybir

    f32 = mybir.dt.float32
    f16 = mybir.dt.float16
    nc = bacc.Bacc("TRN2", target_bir_lowering=False, debug=False)

    # packed inputs (fp32): rows 32*q+e hold f-quarter q = f//16:
    #   featT4[32q+e, (f%16)*64+b] | wstat4[32q+e, (f%16)*32+d]
    c32 = nc.dram_tensor(
        "c32", (128, 16 * BLOC + 16 * E), f32, kind="ExternalInput"
    ).ap()
    c16 = nc.dram_tensor(
        "c16", (128, 16 * F + 2 * GD), f16, kind="ExternalInput"
    ).ap()
    out = nc.dram_tensor("out", (BLOC, F, GD), f32, kind="ExternalOutput").ap()

    with tile.TileContext(nc) as tc, ExitStack() as ctx:
        consts = ctx.enter_context(tc.tile_pool(name="consts", bufs=1))
        # phase-A fp32 operands on sync; fp16 consts on scalar -- 2 DMAs total
        c32_t = consts.tile([128, 16 * BLOC + 16 * E], f32)
        nc.sync.dma_start(c32_t[:], c32)
        c16_t = consts.tile([128, 16 * F + 2 * GD], f16)
        # R-build consts (wlrep|mask) on their own queue so R0 starts early;
        # featd is only needed once the first STT runs
        nc.scalar.dma_start(
            c16_t[:, 0 : 16 * F + GD], c16[:, 0 : 16 * F + GD]
        )
        nc.scalar.dma_start(
            c16_t[:, 16 * F + GD :], c16[:, 16 * F + GD :]
        )
        featT_t = c32_t[:, 0 : 16 * BLOC]
        wstat_t = c32_t[:, 16 * BLOC : 16 * BLOC + 16 * E]
        wlrep_t = c16_t[:, 0 : 16 * F]
        mask_t = c16_t[:, 16 * F : 16 * F + GD]
        featd_t = c16_t[:, 16 * F + GD : 16 * F + 2 * GD]
        # biLinT16[(r,d'), j4*64+b] = biLinear[b, 4*j4+r, d'] * 2^5, fp16
        bilin_t = consts.tile([128, 16 * BLOC], f16)

        # Phase A: biLinT via fp32 matmuls, 4 f's concurrent in col-groups.
        with tc.tile_pool(name="psb", bufs=2, space=bass.MemorySpace.PSUM) as psb:
            for j4 in range(16):
                bps = psb.tile([128, BLOC], f32)
                for r in range(4):
                    f = 4 * j4 + r
                    q, fq = f // 16, f % 16
                    qsl = slice(32 * q, 32 * q + 32)
                    nc.tensor.matmul(
                        bps[32 * r : 32 * r + 32, :],
                        wstat_t[qsl, 32 * fq : 32 * fq + 32],
                        featT_t[qsl, fq * BLOC : (fq + 1) * BLOC],
                        start=True,
                        stop=True,
                        tile_position=(32 * q, 32 * r),
                    )
                # cast to fp16 with 2^5 pre-scale (ACT engine; keeps DVE free)
                nc.scalar.activation(
                    bilin_t[:, j4 * BLOC : (j4 + 1) * BLOC],
                    bps[:],
                    mybir.ActivationFunctionType.Copy,
                    scale=SCALE_BL,
                )

        reng = nc.vector if rbuild == "vector" else nc.gpsimd
        with (
            tc.tile_pool(name="rt", bufs=4) as rtp,
            tc.tile_pool(name="psq", bufs=2, space=bass.MemorySpace.PSUM) as psq,
            tc.tile_pool(name="ot", bufs=6) as otp,
        ):
            mask3 = mask_t.rearrange("p (g d) -> p g d", d=E)
            for j4 in range(16):
                # R[(r,d'),(g,d)] = mask(d'==d) * wL[4j4+r, g] * 2^10  (fp16)
                rt = rtp.tile([128, GD], f16)
                wl3 = (
                    wlrep_t[:, j4 * F : (j4 + 1) * F]
                    .unsqueeze(2)
                    .broadcast_to((128, F, E))
                )
                reng.tensor_tensor(
                    rt[:].rearrange("p (g d) -> p g d", d=E),
                    mask3,
                    wl3,
                    mybir.AluOpType.mult,
                )
                # half-tile (1024-col) PSUM units: pq tags pq0/pq1 x bufs=2
                # ping-pong across cc -> PE never waits on the DVE drain
                for s in range(2):
                    f0 = 4 * j4 + 2 * s
                    eng = nc.sync if s == 0 else nc.scalar
                    for cc in range(2):
                        csl = slice(1024 * cc, 1024 * (cc + 1))
                        pq = psq.tile([128, 1024], f32, name=f"pq{s}", tag=f"pq{s}")
                        for n in range(2):
                            nsl = slice(1024 * cc + 512 * n, 1024 * cc + 512 * (n + 1))
                            for half in range(2):
                                r = 2 * s + half
                                rsl = slice(32 * r, 32 * r + 32)
                                nc.tensor.matmul(
                                    pq[64 * half : 64 * half + 64, 512 * n : 512 * (n + 1)],
                                    bilin_t[rsl, j4 * BLOC : (j4 + 1) * BLOC],
                                    rt[rsl, nsl],
                                    start=True,
                                    stop=True,
                                    tile_position=(32 * r, 64 * half),
                                )
                        ot = otp.tile([128, 1024], f32)
                        # out = (Q / 2^15) * feature
                        nc.vector.scalar_tensor_tensor(
                            ot[:],
                            pq[:],
                            DESCALE,
                            featd_t[:, csl],
                            op0=mybir.AluOpType.mult,
                            op1=mybir.AluOpType.mult,
                        )
                        eng.dma_start(out[:, f0, csl], ot[0:64, :])
                        eng.dma_start(out[:, f0 + 1, csl], ot[64:128, :])

    nc.compile()
    return nc


def _get_nc(rbuild=RBUILD):
    if rbuild not in _cached:
        _cached[rbuild] = _build_nc(rbuild)
    return _cached[rbuild]


def _host_inputs(feature, weight, weightLeft, rbuild=RBUILD):
    """Per-core input maps. Host work is layout prep of weights/inputs only."""
    feature = np.ascontiguousarray(feature, dtype=np.float32)
    weight = np.ascontiguousarray(weight, dtype=np.float32)
    weightLeft = np.ascontiguousarray(weightLeft, dtype=np.float32)

    # wstat[e, f*32+d] = weight[f,e,d]
    wstat = np.ascontiguousarray(weight.transpose(1, 0, 2).reshape(E, F * E))
    # wlrep[32*r+d', j4*64+g] = wL[4*j4+r, g] * 2^10   (d'-replicated)
    wl4 = (weightLeft * np.float32(SCALE_WL)).reshape(16, 4, F)  # [j4, r, g]
    wlrep = np.broadcast_to(
        wl4.transpose(1, 0, 2)[:, None, :, :], (4, E, 16, F)
    ).reshape(128, 16 * F)
    wlrep = np.ascontiguousarray(wlrep.astype(np.float16))
    # maskc[32*r+d', g*32+d] = 1 if d==d' else 0
    maskc = np.ascontiguousarray(np.tile(np.eye(E, dtype=np.float16), (4, F)))

    # f-quarter packing: rows 32*q+e for q = f//16
    wstat4 = (
        wstat.reshape(E, 4, 16 * E).transpose(1, 0, 2).reshape(128, 16 * E)
    )
    in_maps = []
    for c in range(NCORES):
        fc = feature[c * BLOC : (c + 1) * BLOC]  # [64, 64, 32]
        featd = np.tile(fc.reshape(BLOC, GD), (2, 1)).astype(np.float16)
        featT = fc.transpose(2, 1, 0).reshape(E, F * BLOC)
        featT4 = (
            featT.reshape(E, 4, 16 * BLOC)
            .transpose(1, 0, 2)
            .reshape(128, 16 * BLOC)
        )
        c32 = np.ascontiguousarray(np.concatenate([featT4, wstat4], axis=1))
        c16 = np.ascontiguousarray(
            np.concatenate([wlrep, maskc, featd], axis=1)
        )
        in_maps.append({"c32": c32, "c16": c16})
    return in_maps


def _run(in_maps, trace=False, tmpdir=None, rbuild=RBUILD):
    from concourse.bass_utils import run_bass_kernel_spmd

    nc = _get_nc(rbuild)
    return run_bass_kernel_spmd(
        nc, in_maps, core_ids=list(range(NCORES)), trace=trace, tmpdir=tmpdir
    )


def kernel(feature, weight, weightLeft):
    in_maps = _host_inputs(feature, weight, weightLeft)
    res = _run(in_maps)
    out = np.concatenate(
        [r["out"].reshape(BLOC, F, F, E) for r in res.results], axis=0
    )
    return out


# revision 33
# speedup vs baseline: 1.1205x; 1.0213x over previous
"""Trainium2 Bass kernel for BiLinearLayerV2.

  biLinear[b,f,d] = sum_e feature[b,f,e] * weight[f,e,d]
  out[b,f,g,d]    = biLinear[b,f,d] * feature[b,g,d] * weightLeft[f,g]

Shapes: feature [512,64,32] f32, weight [64,32,32], weightLeft [64,64]
Output: [512,64,64,32] f32 (256 MB) -> heavily store-bound.

Data-parallel over batch (64 per core x 8 cores). Per core the output is
33.5 MB, so the kernel lives at the HBM-store roofline; every other byte
of traffic must be squeezed out. Two-stage structure ("kron" design):

  1. biLinT_f[d,b] = sum_e weight[f,e,d] * feature[b,f,e] -- computed
     on-chip in fp32 (exact w.r.t. the cancellation-prone e-sum), tiny.
  2. Q_f[b,(g,d)] = biLinT_f[d,b] * wL[f,g] via PE matmul with moving
     operand R_f[d',(g,d)] = mask(d'==d) * wL[f,g], built ON-CHIP per
     f-group as (constant 0/1 mask) * (broadcast wL row). Both operands
     fp16: stage 2 is a pure product, so fp16 rounding enters
     multiplicatively (no cancellation blow-up) -> rel err ~1e-3.
  3. DVE: out = (Q / 2^15) * feature  (PSUM -> SBUF), stores via HWDGE.

This removes the 16.8 MB fused-weight (wv) HBM load of the naive scheme;
per-core DMA is ~2 MB of inputs + 33.5 MB of stores.

fp16 subnormal guard: wL is pre-scaled by 2^10 and biLinT by 2^5 so all
fp16-carried values stay in the normal range; the final DVE pass undoes
the 2^15.
"""

import sys

if "/opt/trn_rl_repo" not in sys.path:
    sys.path.insert(0, "/opt/trn_rl_repo")

import numpy as np

B, F, E = 512, 64, 32
NCORES = 8
BLOC = B // NCORES  # 64
GD = F * E  # 2048
SCALE_WL = 1024.0  # 2^10 on weightLeft
SCALE_BL = 32.0  # 2^5 on biLinT (fp16 cast)
DESCALE = 1.0 / (SCALE_WL * SCALE_BL)

# engine for the on-chip R = mask * wL broadcast build: "vector" | "gpsimd"
RBUILD = "gpsimd"

_cached = {}


def _build_nc(rbuild=RBUILD):
    from contextlib import ExitStack

    import concourse.bass as bass
    import concourse.tile as tile
    from concourse import bacc, mybir

    f32 = mybir.dt.float32
    f16 = mybir.dt.float16
    nc = bacc.Bacc("TRN2", target_bir_lowering=False, debug=False)

    # packed inputs (fp32): rows 32*q+e hold f-quarter q = f//16:
    #   featT4[32q+e, (f%16)*64+b] | wstat4[32q+e, (f%16)*32+d]
    c32 = nc.dram_tensor(
        "c32", (128, 16 * BLOC + 16 * E), f32, kind="ExternalInput"
    ).ap()
    # c16 = wlrep (1024) | mask32 (32, g-broadcast on chip) | featd (2048)
    c16 = nc.dram_tensor(
        "c16", (128, 16 * F + E + GD), f16, kind="ExternalInput"
    ).ap()
    out = nc.dram_tensor("out", (BLOC, F, GD), f32, kind="ExternalOutput").ap()

    with tile.TileContext(nc) as tc, ExitStack() as ctx:
        consts = ctx.enter_context(tc.tile_pool(name="consts", bufs=1))
        # phase-A fp32 operands on sync; fp16 consts on scalar -- 2 DMAs total
        c32_t = consts.tile([128, 16 * BLOC + 16 * E], f32)
        c16_t = consts.tile([128, 16 * F + E + GD], f16)
        # R-build consts (wlrep|mask32, 0.26 MB) first on scalar so R0 starts
        # ASAP; c32 halves split across both queues (separate deps let
        # phase A j4 0-7 start on the sync half alone); featd last.
        nc.sync.dma_start(c32_t[0:64, :], c32[0:64, :])
        nc.scalar.dma_start(
            c16_t[:, 0 : 16 * F + E], c16[:, 0 : 16 * F + E]
        )
        nc.scalar.dma_start(c32_t[64:128, :], c32[64:128, :])
        nc.sync.dma_start(
            c16_t[:, 16 * F + E :], c16[:, 16 * F + E :]
        )
        featT_t = c32_t[:, 0 : 16 * BLOC]
        wstat_t = c32_t[:, 16 * BLOC : 16 * BLOC + 16 * E]
        wlrep_t = c16_t[:, 0 : 16 * F]
        mask_t = c16_t[:, 16 * F : 16 * F + E]
        featd_t = c16_t[:, 16 * F + E : 16 * F + E + GD]
        # biLinT16[(r,d'), j4*64+b] = biLinear[b, 4*j4+r, d'] * 2^5, fp16
        bilin_t = consts.tile([128, 16 * BLOC], f16)

        # Phase A: biLinT via fp32 matmuls, 4 f's concurrent in col-groups.
        with tc.tile_pool(name="psb", bufs=2, space=bass.MemorySpace.PSUM) as psb:
            for j4 in range(16):
                bps = psb.tile([128, BLOC], f32)
                for r in range(4):
                    f = 4 * j4 + r
                    q, fq = f // 16, f % 16
                    qsl = slice(32 * q, 32 * q + 32)
                    nc.tensor.matmul(
                        bps[32 * r : 32 * r + 32, :],
                        wstat_t[qsl, 32 * fq : 32 * fq + 32],
                        featT_t[qsl, fq * BLOC : (fq + 1) * BLOC],
                        start=True,
                        stop=True,
                        tile_position=(32 * q, 32 * r),
                    )
                # cast to fp16 with 2^5 pre-scale (ACT engine; keeps DVE free)
                nc.scalar.activation(
                    bilin_t[:, j4 * BLOC : (j4 + 1) * BLOC],
                    bps[:],
                    mybir.ActivationFunctionType.Copy,
                    scale=SCALE_BL,
                )

        reng = nc.vector if rbuild == "vector" else nc.gpsimd
        with (
            tc.tile_pool(name="rt", bufs=6) as rtp,
            tc.tile_pool(name="psq", bufs=2, space=bass.MemorySpace.PSUM) as psq,
            tc.tile_pool(name="ot", bufs=8) as otp,
        ):
            # mask32 [128,32] broadcast over g via stride-0 AP
            mask3 = mask_t.unsqueeze(1).broadcast_to((128, F, E))
            for j4 in range(16):
                # R[(r,d'),(g,d)] = mask(d'==d) * wL[4j4+r, g] * 2^10  (fp16)
                rt = rtp.tile([128, GD], f16)
                wl3 = (
                    wlrep_t[:, j4 * F : (j4 + 1) * F]
                    .unsqueeze(2)
                    .broadcast_to((128, F, E))
                )
                reng.tensor_tensor(
                    rt[:].rearrange("p (g d) -> p g d", d=E),
                    mask3,
                    wl3,
                    mybir.AluOpType.mult,
                )
                # half-tile (1024-col) PSUM units: pq tags pq0/pq1 x bufs=2
                # ping-pong across cc -> PE never waits on the DVE drain
                for s in range(2):
                    f0 = 4 * j4 + 2 * s
                    eng = nc.sync if s == 0 else nc.scalar
                    for cc in range(2):
                        csl = slice(1024 * cc, 1024 * (cc + 1))
                        pq = psq.tile([128, 1024], f32, name=f"pq{s}", tag=f"pq{s}")
                        for n in range(2):
                            nsl = slice(1024 * cc + 512 * n, 1024 * cc + 512 * (n + 1))
                            for half in range(2):
                                r = 2 * s + half
                                rsl = slice(32 * r, 32 * r + 32)
                                nc.tensor.matmul(
                                    pq[64 * half : 64 * half + 64, 512 * n : 512 * (n + 1)],
                                    bilin_t[rsl, j4 * BLOC : (j4 + 1) * BLOC],
                                    rt[rsl, nsl],
                                    start=True,
                                    stop=True,
                                    tile_position=(32 * r, 64 * half),
                                )
                        ot = otp.tile([128, 1024], f32)
                        # out = (Q / 2^15) * feature
                        nc.vector.scalar_tensor_tensor(
                            ot[:],
                            pq[:],
                            DESCALE,
                            featd_t[:, csl],
                            op0=mybir.AluOpType.mult,
                            op1=mybir.AluOpType.mult,
                        )
                        eng.dma_start(out[:, f0, csl], ot[0:64, :])
                        eng.dma_start(out[:, f0 + 1, csl], ot[64:128, :])

    nc.compile()
    return nc


def _get_nc(rbuild=RBUILD):
    if rbuild not in _cached:
        _cached[rbuild] = _build_nc(rbuild)
    return _cached[rbuild]


def _host_inputs(feature, weight, weightLeft, rbuild=RBUILD):
    """Per-core input maps. Host work is layout prep of weights/inputs only."""
    feature = np.ascontiguousarray(feature, dtype=np.float32)
    weight = np.ascontiguousarray(weight, dtype=np.float32)
    weightLeft = np.ascontiguousarray(weightLeft, dtype=np.float32)

    # wstat[e, f*32+d] = weight[f,e,d]
    wstat = np.ascontiguousarray(weight.transpose(1, 0, 2).reshape(E, F * E))
    # wlrep[32*r+d', j4*64+g] = wL[4*j4+r, g] * 2^10   (d'-replicated)
    wl4 = (weightLeft * np.float32(SCALE_WL)).reshape(16, 4, F)  # [j4, r, g]
    wlrep = np.broadcast_to(
        wl4.transpose(1, 0, 2)[:, None, :, :], (4, E, 16, F)
    ).reshape(128, 16 * F)
    wlrep = np.ascontiguousarray(wlrep.astype(np.float16))
    # mask32[32*r+d', d] = 1 if d==d' else 0 (g-dim broadcast on chip)
    maskc = np.ascontiguousarray(np.tile(np.eye(E, dtype=np.float16), (4, 1)))

    # f-quarter packing: rows 32*q+e for q = f//16
    wstat4 = (
        wstat.reshape(E, 4, 16 * E).transpose(1, 0, 2).reshape(128, 16 * E)
    )
    in_maps = []
    for c in range(NCORES):
        fc = feature[c * BLOC : (c + 1) * BLOC]  # [64, 64, 32]
        featd = np.tile(fc.reshape(BLOC, GD), (2, 1)).astype(np.float16)
        featT = fc.transpose(2, 1, 0).reshape(E, F * BLOC)
        featT4 = (
            featT.reshape(E, 4, 16 * BLOC)
            .transpose(1, 0, 2)
            .reshape(128, 16 * BLOC)
        )
        c32 = np.ascontiguousarray(np.concatenate([featT4, wstat4], axis=1))
        c16 = np.ascontiguousarray(
            np.concatenate([wlrep, maskc, featd], axis=1)
        )
        in_maps.append({"c32": c32, "c16": c16})
    return in_maps


def _run(in_maps, trace=False, tmpdir=None, rbuild=RBUILD):
    from concourse.bass_utils import run_bass_kernel_spmd

    nc = _get_nc(rbuild)
    return run_bass_kernel_spmd(
        nc, in_maps, core_ids=list(range(NCORES)), trace=trace, tmpdir=tmpdir
    )


def kernel(feature, weight, weightLeft):
    in_maps = _host_inputs(feature, weight, weightLeft)
    res = _run(in_maps)
    out = np.concatenate(
        [r["out"].reshape(BLOC, F, F, E) for r in res.results], axis=0
    )
    return out
